# revision 18
# baseline (speedup 1.0000x reference)
"""Multi-head attention (B=4,S=2048,D=1024,H=16,dh=65) on 8 TRN2 NeuronCores.

Sharding: batch x head-half. Core c handles batch c//2 and heads
(c%2)*8..(c%2)*8+8 (P-slice of 520). Each core computes its QKV projections,
attention, and a partial out-projection; the host sums the two partials per
batch and adds bo.

v3: PE-array packing via tile_position.
 - K/Q projections stay in pair-packed [headA dh0-63 | headB dh0-63] form
   (rows 0-63 / 64-127 of a 128-partition tile) -- no post-projection DMA
   shuffle. Projections run in fp8 DoubleRow (weights x64 pre-scaled, inputs
   fp8) for ~1.8x PE throughput; the un-scale rides the bias tensor_scalar.
 - Scores: two concurrent row-tiled K=64 matmuls per ktile (tile_position
   rows 0/64) + per-head 2-packed rank-1 straggler matmuls for the dh64 term
   (tile_position rows 0/32 and 64/96, K=1).
 - Softmax: per-head [128, 2, 512] exp on ScalarE (the ~293us bottleneck this
   schedule is built around), per-head mask multiply on DVE/GpSimd.
 - AV: two concurrent col-tiled M=64 matmuls per ktile (tile_position cols
   0/64) into one PSUM bank + a 4-packed M=2 straggler slot per round
   (dh64 + ones-row-sum, cols 0/32/64/96 of a second bank).
 - Out-projection stationary = pair-major concat rows (woT reordered on
   host); output written bf16 and upcast on host.
"""

import math
import sys
from collections import deque

import numpy as np
import ml_dtypes

sys.path.insert(0, "/opt/trn_rl_repo")

import concourse.bass as bass
import concourse.mybir as mybir
import concourse.tile as tile_mod
from concourse.bass_utils import run_bass_kernel_spmd
from concourse.vector_clock import ScopedClock

# ---------------------------------------------------------------------------
# Patch for this container's walrus build: it rejects instructions carrying
# more than one semaphore wait ("Too many sync wait commands"), but Tile's
# wait assigner freely attaches several. Split excess waits onto bass_nofuse
# InstNoOp carriers on the same engine, committed immediately before the
# instruction (same-engine program order => over-synchronization only).
# ---------------------------------------------------------------------------
_MAX_WAITS = 1

_orig_commit = tile_mod.TileContext._commit_instruction


def _split_waits(self, inst, commit):
    si = inst.sync_info
    if si is None or len(si.on_wait) <= _MAX_WAITS:
        return
    waits = list(si.on_wait)
    sem_w = [w for w in waits if getattr(w, "sync_type", "semaphore") == "semaphore"]
    other_w = [w for w in waits if getattr(w, "sync_type", "semaphore") != "semaphore"]
    keep_budget = _MAX_WAITS - len(other_w)
    if keep_budget < 0:
        return
    keep = other_w + (sem_w[-keep_budget:] if keep_budget > 0 else [])
    excess = sem_w[: len(sem_w) - max(keep_budget, 0)]
    if not excess:
        return
    for i, w in enumerate(excess):
        nop = mybir.InstNoOp(
            name=f"{inst.name}-sw{i}",
            sync_info=mybir.SyncInfo(on_wait=[w], on_update=[]),
            bass_nofuse=True,
            engine=inst.engine,
        )
        commit(nop)
    inst.sync_info = mybir.SyncInfo(on_wait=keep, on_update=list(si.on_update))


def _patched_commit(self, inst, lazy_reg_writes: bool = True):
    if inst.engine != mybir.EngineType.Unassigned:
        _split_waits(self, inst, lambda n: _orig_commit(self, n, False))
    return _orig_commit(self, inst, lazy_reg_writes)


def _patched_drain_and_barrier(self, tick_clock, wait_clock):
    drain_inst = self.nc.sync.drain()
    wait_clock.add_sem_waits(
        drain_inst.ins, ScopedClock({None: tick_clock.global_clock})
    )
    si = drain_inst.ins.sync_info
    if si is not None and len(si.on_wait) > _MAX_WAITS:
        waits = list(si.on_wait)
        drain_inst.ins.sync_info = mybir.SyncInfo(
            on_wait=waits[:_MAX_WAITS], on_update=list(si.on_update)
        )
        for w in waits[_MAX_WAITS:]:
            n = self.nc.sync.nop(nofuse=True)
            n.ins.sync_info = mybir.SyncInfo(on_wait=[w], on_update=[])
    self.nc.all_engine_barrier()
    popped = self.nc._tile_sem_poison_stack.pop()
    assert popped is self._sem_poison
    self.nc.clear_and_free_semaphores(list(self.sems.allocated().values()))
    self.nc.all_engine_barrier()


tile_mod.TileContext._commit_instruction = _patched_commit
tile_mod.TileContext._drain_and_barrier = _patched_drain_and_barrier

# ---------------------------------------------------------------------------

B, S, D, H = 4, 2048, 1024, 16
DH = D // H + 1          # 65
P = H * DH               # 1040
HPC = H // 2             # heads per core
PC = HPC * DH            # 520, per-core P slice
NPAIR = HPC // 2         # 4 head pairs per core
N_CORES = 8

MT = S // 128            # 16 row blocks / k tiles
KT = 16                  # k tiles per attention
QB = 4                   # q blocks of 512
QW = 512
NR = 8                   # rounds (ktile pairs) per (pair, qb)
AV_LAG = 2               # AV closures lag the score stream by this many slots

F32 = mybir.dt.float32
BF16 = mybir.dt.bfloat16
FP8 = mybir.dt.float8e4
BF = ml_dtypes.bfloat16
F8 = ml_dtypes.float8_e4m3
W8SCALE = 64.0

_BUILT = {}


def _build_nc():
    nc = bass.Bass("TRN2", target_bir_lowering=False, debug=False,
                   num_devices=N_CORES)
    MUL = mybir.AluOpType.mult
    ADD = mybir.AluOpType.add
    DR = mybir.MatmulPerfMode.DoubleRow

    # fp8 inputs for Q/K projections, d-pair interleaved [dp][128, 2, S]
    xq8_d = nc.dram_tensor("xq8", [4, 128, 2, S], FP8, kind="ExternalInput").ap()
    xk8_d = nc.dram_tensor("xk8", [4, 128, 2, S], FP8, kind="ExternalInput").ap()
    xv_d = nc.dram_tensor("xv", [D, S], BF16, kind="ExternalInput").ap()
    # maskH[qb, p, j*QW+q] = maskT[j*128+p, qb*512+q] (multiplicative 0/1)
    mh = nc.dram_tensor("maskH", [QB, 128, KT * QW], BF16,
                        kind="ExternalInput").ap()
    # pair-packed K/Q weights (x64, fp8, d-pair interleave):
    # [dp][128, 2, pair, 128]; straggler dh64 rows [dp][128, 2, 8]
    wq8_d = nc.dram_tensor("wq8", [4, 128, 2, 4, 128], FP8,
                           kind="ExternalInput").ap()
    wk8_d = nc.dram_tensor("wk8", [4, 128, 2, 4, 128], FP8,
                           kind="ExternalInput").ap()
    # straggler dh64 weights padded to 16 cols (fp8 DoubleRow needs the
    # Ko-dim byte step to be a multiple of 16)
    wq648_d = nc.dram_tensor("wq648", [4, 128, 2, 16], FP8,
                             kind="ExternalInput").ap()
    wk648_d = nc.dram_tensor("wk648", [4, 128, 2, 16], FP8,
                             kind="ExternalInput").ap()
    bqP_d = nc.dram_tensor("bqP", [128, 4], F32, kind="ExternalInput").ap()
    bkP_d = nc.dram_tensor("bkP", [128, 4], F32, kind="ExternalInput").ap()
    bq64_d = nc.dram_tensor("bq64", [HPC, 1], F32, kind="ExternalInput").ap()
    bk64_d = nc.dram_tensor("bk64", [HPC, 1], F32, kind="ExternalInput").ap()
    # V weights packed 8 heads x dh0..63 + dh64 straggler (bf16)
    wvP_d = nc.dram_tensor("wvP", [D, 512], BF16, kind="ExternalInput").ap()
    wv64_d = nc.dram_tensor("wv64", [D, HPC], BF16, kind="ExternalInput").ap()
    bvP_d = nc.dram_tensor("bvP", [1, 512], BF16, kind="ExternalInput").ap()
    bv64_d = nc.dram_tensor("bv64", [1, HPC], BF16, kind="ExternalInput").ap()
    # out-proj stationary, pair-major rows: [512, D] + dh64 rows [8, D]
    woP_d = nc.dram_tensor("woP", [512, D], BF16, kind="ExternalInput").ap()
    wo4_d = nc.dram_tensor("wo4", [HPC, D], BF16, kind="ExternalInput").ap()
    sel2_d = nc.dram_tensor("sel2", [2, 128], BF16, kind="ExternalInput").ap()
    out = nc.dram_tensor("out", [S, D], BF16, kind="ExternalOutput").ap()

    inv_sqrt = 1.0 / math.sqrt(float(DH))

    from contextlib import ExitStack
    with tile_mod.TileContext(nc) as tc:
        with tc.tile_pool(name="const", bufs=1) as pconst, \
             tc.tile_pool(name="kqT", bufs=1) as pkq, \
             tc.tile_pool(name="str", bufs=1) as pstr, \
             tc.tile_pool(name="strq4", bufs=2) as pstrq, \
             tc.tile_pool(name="vh", bufs=MT) as pvh, \
             tc.tile_pool(name="pm", bufs=2) as pm, \
             tc.tile_pool(name="psS", bufs=1, space="PSUM") as psS, \
             tc.tile_pool(name="psA", bufs=1, space="PSUM") as psA, \
             tc.tile_pool(name="psAS", bufs=1, space="PSUM") as psAS, \
             tc.tile_pool(name="psO", bufs=2, space="PSUM") as psO:
            _xw = ExitStack()
            px = _xw.enter_context(tc.tile_pool(name="px", bufs=8))
            pw = _xw.enter_context(tc.tile_pool(name="pw", bufs=1))
            pst8 = _xw.enter_context(tc.tile_pool(name="pst8", bufs=1))

            ones_col = pconst.tile([1, 128], BF16, tag="ones")
            nc.gpsimd.memset(ones_col[:], 1.0)
            # preload the exp table set during the projection phase
            scratch_e = pconst.tile([1, 128], BF16, tag="scratch_e")
            nc.scalar.activation(scratch_e[:], ones_col[:],
                                 mybir.ActivationFunctionType.Exp)
            # sel2: broadcast [2,512] recips to [128,512] pair layout
            sel2 = pconst.tile([2, 128], BF16, tag="sel2")
            nc.gpsimd.dma_start(sel2[:], sel2_d)

            # K/Q transposed pair tiles [128, S]: rows 0-63 head 2p dh0-63,
            # rows 64-127 head 2p+1 dh0-63
            kT = [pkq.tile([128, S], BF16, tag=f"kT{p}", name=f"kT{p}")
                  for p in range(NPAIR)]
            qT = [pkq.tile([128, S], BF16, tag=f"qT{p}", name=f"qT{p}")
                  for p in range(NPAIR)]
            # straggler k64 rows, duplicated per row-group:
            # [:, p, :]: part 0,32 = k64(head 2p), part 64,96 = k64(head 2p+1)
            strk = pstr.tile([128, NPAIR, S], BF16, tag="strk")
            # compact straggler sources [8, S]
            st8k = pst8.tile([HPC, S], BF16, tag="st8k")
            st8q = pstr.tile([HPC, S], BF16, tag="st8q")

            # v k-tiles with trailing ones column: [128, head, 65+1]
            vh = [pvh.tile([128, HPC, DH + 1], BF16, tag="vh", name=f"vh{j}")
                  for j in range(MT)]
            for j in range(MT):
                nc.gpsimd.memset(vh[j][:, :, DH:DH + 1], 1.0)

            # ---------------- input DMAs ------------------
            xk8 = [px.tile([128, 2, S], FP8, tag="x", name=f"xk8{dp}")
                   for dp in range(4)]
            for dp in range(4):
                nc.sync.dma_start(xk8[dp][:], xk8_d[dp])
            wk8 = [pw.tile([128, 2, 4, 128], FP8, tag=f"wk8{dp}",
                           name=f"wk8{dp}") for dp in range(4)]
            wq8 = [pw.tile([128, 2, 4, 128], FP8, tag=f"wq8{dp}",
                           name=f"wq8{dp}") for dp in range(4)]
            wk648 = [pw.tile([128, 2, 16], FP8, tag=f"wk648{dp}",
                             name=f"wk648{dp}") for dp in range(4)]
            wq648 = [pw.tile([128, 2, 16], FP8, tag=f"wq648{dp}",
                             name=f"wq648{dp}") for dp in range(4)]
            for dp in range(4):
                nc.gpsimd.dma_start(wk8[dp][:], wk8_d[dp])
                nc.gpsimd.dma_start(wk648[dp][:], wk648_d[dp])
            bkP_t = pconst.tile([128, 4], F32, tag="bkP")
            bqP_t = pconst.tile([128, 4], F32, tag="bqP")
            bk64_t = pconst.tile([HPC, 1], F32, tag="bk64")
            bq64_t = pconst.tile([HPC, 1], F32, tag="bq64")
            nc.gpsimd.dma_start(bkP_t[:], bkP_d)
            nc.gpsimd.dma_start(bqP_t[:], bqP_d)
            nc.gpsimd.dma_start(bk64_t[:], bk64_d)
            nc.gpsimd.dma_start(bq64_t[:], bq64_d)

            # ---------------- K projection (fp8 DoubleRow) ----------------
            def kq_proj(w8, w648, x8, bP, b64, kqt, st8, tag):
                for p in range(NPAIR):
                    for c in range(4):
                        ps = psO.tile([128, QW], F32, tag="po",
                                      name=f"ps{tag}{p}_{c}")
                        for dp in range(4):
                            nc.tensor.matmul(
                                ps[:], w8[dp][:, :, p, :],
                                x8[dp][:, :, c * QW:(c + 1) * QW],
                                start=(dp == 0), stop=(dp == 3),
                                perf_mode=DR)
                        nc.vector.tensor_scalar(
                            kqt[p][:, c * QW:(c + 1) * QW], ps[:],
                            1.0 / W8SCALE, bP[:, p:p + 1], MUL, ADD)
                for c in range(4):
                    ps8 = psO.tile([128, QW], F32, tag="po",
                                   name=f"ps8{tag}{c}")
                    for dp in range(4):
                        nc.tensor.matmul(
                            ps8[0:16, :], w648[dp][:],
                            x8[dp][:, :, c * QW:(c + 1) * QW],
                            start=(dp == 0), stop=(dp == 3), perf_mode=DR)
                    nc.vector.tensor_scalar(
                        st8[:, c * QW:(c + 1) * QW], ps8[0:HPC, :],
                        1.0 / W8SCALE, b64[:, 0:1], MUL, ADD)

            kq_proj(wk8, wk648, xk8, bkP_t, bk64_t, kT, st8k, "k")
            # scatter k64 rows: partition 0 = head 2p, partition 32 = 2p+1
            for p in range(NPAIR):
                for a in range(2):           # head-in-pair
                    nc.gpsimd.dma_start(
                        strk[32 * a:32 * a + 1, p, :],
                        st8k[2 * p + a:2 * p + a + 1, :])

            xq8 = [px.tile([128, 2, S], FP8, tag="x", name=f"xq8{dp}")
                   for dp in range(4)]
            for dp in range(4):
                nc.sync.dma_start(xq8[dp][:], xq8_d[dp])
                nc.gpsimd.dma_start(wq8[dp][:], wq8_d[dp])
                nc.gpsimd.dma_start(wq648[dp][:], wq648_d[dp])
            kq_proj(wq8, wq648, xq8, bqP_t, bq64_t, qT, st8q, "q")

            # ---------------- V projection (bf16, s-major) ----------------
            wvP_t = []
            wv64_t = []
            for d in range(8):
                wt = pw.tile([128, 512], BF16, tag="wvP", bufs=8,
                             name=f"wvP{d}")
                nc.gpsimd.dma_start(wt[:], wvP_d[d * 128:(d + 1) * 128])
                wvP_t.append(wt)
                w64t = pw.tile([128, HPC], BF16, tag="wv64", bufs=8,
                               name=f"wv64{d}")
                nc.gpsimd.dma_start(w64t[:], wv64_d[d * 128:(d + 1) * 128])
                wv64_t.append(w64t)
            bvP_t = pconst.tile([1, 512], BF16, tag="bvP")
            nc.gpsimd.dma_start(bvP_t[:], bvP_d)
            bv64_t = pconst.tile([1, HPC], BF16, tag="bv64")
            nc.gpsimd.dma_start(bv64_t[:], bv64_d)
            xv_t = [px.tile([128, S], BF16, tag="x", name=f"xv{d}")
                    for d in range(8)]
            for c in range(2):
                for d in range(8):
                    nc.sync.dma_start(
                        xv_t[d][:, c * 1024:(c + 1) * 1024],
                        xv_d[d * 128:(d + 1) * 128,
                             c * 1024:(c + 1) * 1024])

            for m in range(MT):
                psa = psA.tile([128, QW], F32, tag="av", name=f"psva{m}")
                psb = psO.tile([128, QW], F32, tag="po", name=f"psvb{m}")
                for d in range(8):
                    nc.tensor.matmul(
                        psa[:], xv_t[d][:, m * 128:(m + 1) * 128],
                        wvP_t[d][:], start=(d == 0), stop=False)
                    nc.tensor.matmul(
                        psb[:, 0:HPC], xv_t[d][:, m * 128:(m + 1) * 128],
                        wv64_t[d][:], start=(d == 0), stop=False)
                nc.tensor.matmul(psa[:], ones_col[0:1, :],
                                 bvP_t[0:1, :], start=False, stop=True)
                nc.tensor.matmul(psb[:, 0:HPC], ones_col[0:1, :],
                                 bv64_t[0:1, :], start=False, stop=True)
                nc.vector.tensor_copy(vh[m][:, :, 0:64], psa[:])
                nc.vector.tensor_copy(vh[m][:, :, 64:65], psb[:, 0:HPC])

            _xw.close()   # free x/w pools before attention phase

            # ---------------- attention + out-projection -------------------
            _at = ExitStack()
            pp = _at.enter_context(tc.tile_pool(name="pp", bufs=8))
            pc = _at.enter_context(tc.tile_pool(name="pc", bufs=1))
            pwo = _at.enter_context(tc.tile_pool(name="pwo", bufs=1))
            pe = _at.enter_context(tc.tile_pool(name="pe", bufs=1))
            # concat tiles, pair-major + dh64 rows
            ccp = [pc.tile([128, S], BF16, tag=f"ccp{p}", name=f"ccp{p}")
                   for p in range(NPAIR)]
            cc4 = pc.tile([HPC, S], BF16, tag="cc4")
            wop = []
            for p in range(NPAIR):
                w = pwo.tile([128, D], BF16, tag=f"wop{p}", name=f"wop{p}")
                nc.gpsimd.dma_start(w[:], woP_d[p * 128:(p + 1) * 128, :])
                wop.append(w)
            wo4t = pwo.tile([HPC, D], BF16, tag="wo4t")
            nc.gpsimd.dma_start(wo4t[:], wo4_d)
            pending = deque()
            bgq = deque()

            def drain(nbg=1):
                pops = 0
                while len(pending) > AV_LAG and pops < 4:
                    pending.popleft()()
                    pops += 1
                for _ in range(nbg):
                    if bgq:
                        bgq.popleft()()

            def mk_av(qb, p, j, pt, ctx):
                def cl():
                    if "av" not in ctx:
                        ctx["av"] = psA.tile([128, QW], F32, tag="av",
                                             name=f"av{qb}_{p}")
                    av = ctx["av"]
                    nc.tensor.matmul(
                        av[0:64, :], vh[j][:, 2 * p, 0:64],
                        pt[:, 0, :], start=(j == 0), stop=(j == KT - 1),
                        tile_position=(0, 0))
                    nc.tensor.matmul(
                        av[64:128, :], vh[j][:, 2 * p + 1, 0:64],
                        pt[:, 1, :], start=(j == 0), stop=(j == KT - 1),
                        tile_position=(0, 64))
                return cl

            def mk_avs(qb, p, j0, pt0, pt1, ctx):
                # stragglers: 4-pack (dh64, ones) x (head, ktile parity)
                def cl():
                    if "avs" not in ctx:
                        ctx["avs"] = psAS.tile([128, QW], F32, tag="avs",
                                               name=f"avs{qb}_{p}")
                    avs = ctx["avs"]
                    for idx, (hh, par) in enumerate(
                            ((0, 0), (0, 1), (1, 0), (1, 1))):
                        pt = pt0 if par == 0 else pt1
                        nc.tensor.matmul(
                            avs[32 * idx:32 * idx + 2, :],
                            vh[j0 + par][:, 2 * p + hh, 64:66],
                            pt[:, hh, :], start=(j0 == 0),
                            stop=(j0 == KT - 2),
                            tile_position=(0, 32 * idx))
                return cl

            def mk_evac(qb, p, ctx):
                # returns a list of closures
                cls = []

                def c_uov():
                    ctx["uov"] = pe.tile([128, QW], BF16, tag="uov", bufs=3,
                                         name=f"uov{qb}_{p}")
                    nc.vector.tensor_copy(ctx["uov"][:], ctx["av"][:])

                def c_sst():
                    ctx["sst"] = pe.tile([128, QW], BF16, tag="sst", bufs=2,
                                         name=f"sst{qb}_{p}")
                    nc.vector.tensor_copy(ctx["sst"][:], ctx["avs"][:])

                def c_gather():
                    # per-quantity tiles [head, parity, q] so every engine op
                    # reads from partition base 0 (BIR alignment rule)
                    orx_o = pe.tile([2, 2, QW], BF16, tag="orx_o", bufs=2,
                                    name=f"orxo{qb}_{p}")
                    orx_r = pe.tile([2, 2, QW], BF16, tag="orx_r", bufs=2,
                                    name=f"orxr{qb}_{p}")
                    ctx["orx_o"] = orx_o
                    ctx["orx_r"] = orx_r
                    sst = ctx["sst"]
                    # avs idx layout: 0:(A,ev)@0, 1:(A,od)@32, 2:(B,ev)@64,
                    # 3:(B,od)@96 ; row 32*idx = o64, row 32*idx+1 = rowsum
                    for idx, (hh, par) in enumerate(
                            ((0, 0), (0, 1), (1, 0), (1, 1))):
                        nc.sync.dma_start(
                            orx_o[hh:hh + 1, par, :],
                            sst[32 * idx:32 * idx + 1, :])
                        nc.sync.dma_start(
                            orx_r[hh:hh + 1, par, :],
                            sst[32 * idx + 1:32 * idx + 2, :])

                def c_sum():
                    osum = pe.tile([2, QW], F32, tag="osum", bufs=2,
                                   name=f"osum{qb}_{p}")
                    rsum = pe.tile([2, QW], F32, tag="rsum", bufs=2,
                                   name=f"rsum{qb}_{p}")
                    ctx["osum"] = osum
                    ctx["rsum"] = rsum
                    nc.vector.tensor_add(osum[:], ctx["orx_o"][:, 0, :],
                                         ctx["orx_o"][:, 1, :])
                    nc.vector.tensor_add(rsum[:], ctx["orx_r"][:, 0, :],
                                         ctx["orx_r"][:, 1, :])

                def c_recip():
                    rc4 = pe.tile([2, QW], F32, tag="rc4", bufs=2,
                                  name=f"rc4{qb}_{p}")
                    ctx["rc4"] = rc4
                    nc.vector.reciprocal(rc4[:], ctx["rsum"][:])

                def c_rcast():
                    rcb = pe.tile([2, QW], BF16, tag="rcb", bufs=2,
                                  name=f"rcb{qb}_{p}")
                    ctx["rcb"] = rcb
                    nc.vector.tensor_copy(rcb[:], ctx["rc4"][:])

                def c_rbp():
                    rbp = psO.tile([128, QW], F32, tag="po",
                                   name=f"rbp{qb}_{p}")
                    ctx["rbp"] = rbp
                    nc.tensor.matmul(rbp[:], sel2[:], ctx["rcb"][:],
                                     start=True, stop=True)

                def c_cch():
                    nc.vector.tensor_mul(
                        ccp[p][:, qb * QW:(qb + 1) * QW],
                        ctx["uov"][:], ctx["rbp"][:])

                def c_o64():
                    o64n = pe.tile([2, QW], BF16, tag="o64n", bufs=2,
                                   name=f"o64n{qb}_{p}")
                    ctx["o64n"] = o64n
                    nc.vector.tensor_mul(o64n[:], ctx["osum"][:],
                                         ctx["rcb"][:])

                def c_o64dma():
                    nc.sync.dma_start(
                        cc4[2 * p:2 * p + 2, qb * QW:(qb + 1) * QW],
                        ctx["o64n"][:])

                cls.extend([c_uov, c_sst, c_gather, c_sum, c_recip,
                            c_rcast, c_rbp, c_cch, c_o64, c_o64dma])
                return cls

            def mk_outproj(qb):
                cls = []

                def mk_mm(m, n, i, ctx):
                    def cl():
                        if n not in ctx:
                            ctx[n] = psO.tile([128, QW], F32, tag="po",
                                              name=f"pso{m}_{n}")
                        src = ccp[i] if i < NPAIR else cc4
                        rows = 128 if i < NPAIR else HPC
                        wo = wop[i] if i < NPAIR else wo4t
                        nc.tensor.matmul(
                            ctx[n][:], src[0:rows, m * 128:(m + 1) * 128],
                            wo[:, n * QW:(n + 1) * QW],
                            start=(i == 0), stop=(i == NPAIR))
                    return cl

                def mk_copy(m, n, ctx):
                    def cl():
                        if "osb" not in ctx:
                            ctx["osb"] = pe.tile([128, D], BF16,
                                                 tag="osb", bufs=2,
                                                 name=f"osb{m}")
                        nc.vector.tensor_copy(
                            ctx["osb"][:, n * QW:(n + 1) * QW],
                            ctx[n][:])
                    return cl

                def mk_dma(m, ctx):
                    def cl():
                        nc.sync.dma_start(
                            out[m * 128:(m + 1) * 128, :], ctx["osb"][:])
                    return cl

                for m in range(qb * 4, (qb + 1) * 4):
                    ctx = {}
                    for n in (0, 1):
                        for i in range(NPAIR + 1):
                            cls.append(mk_mm(m, n, i, ctx))
                        cls.append(mk_copy(m, n, ctx))
                    cls.append(mk_dma(m, ctx))
                return cls

            mts = {}
            sq4s = {}

            def ensure_qb(qb):
                if qb in mts or qb >= QB:
                    return
                mt = pm.tile([128, KT, QW], BF16, tag="mask",
                             name=f"mask{qb}")
                nc.sync.dma_start(mt[:], mh[qb])
                mts[qb] = mt
                # per-qb straggler q rows [g*32, p, :] with dup
                sq4 = pstrq.tile([128, NPAIR, QW], BF16, tag="sq4",
                                 name=f"sq4{qb}")
                for p in range(NPAIR):
                    for a in range(2):
                        nc.gpsimd.dma_start(
                            sq4[32 * a:32 * a + 1, p, :],
                            st8q[2 * p + a:2 * p + a + 1,
                                 qb * QW:(qb + 1) * QW])
                sq4s[qb] = sq4

            for qb in range(QB):
                ensure_qb(qb)
                mt = mts[qb]
                sq4 = sq4s[qb]

                for p in range(NPAIR):
                    if p == 1 and qb > 0:
                        # previous qb's evac has fully drained by now
                        bgq.extend(mk_outproj(qb - 1))
                    if p == 2:
                        ensure_qb(qb + 1)
                    ctx = {}
                    prev_pt = None
                    for j in range(KT):
                        ss = psS.tile([128, 2, QW], F32, tag="ss", bufs=2,
                                      name=f"ss{qb}_{p}_{j}")
                        nc.tensor.matmul(
                            ss[:, 0, :], kT[p][0:64, j * 128:(j + 1) * 128],
                            qT[p][0:64, qb * QW:(qb + 1) * QW],
                            start=True, stop=False, tile_position=(0, 0))
                        nc.tensor.matmul(
                            ss[:, 1, :], kT[p][64:128, j * 128:(j + 1) * 128],
                            qT[p][64:128, qb * QW:(qb + 1) * QW],
                            start=True, stop=False, tile_position=(64, 0))
                        # rank-1 dh64 stragglers, 2-packed (rows 0 / 32)
                        nc.tensor.matmul(
                            ss[:, 0, :],
                            strk[0:1, p, j * 128:(j + 1) * 128],
                            sq4[0:1, p, :], start=False, stop=True,
                            tile_position=(0, 0))
                        nc.tensor.matmul(
                            ss[:, 1, :],
                            strk[32:33, p, j * 128:(j + 1) * 128],
                            sq4[32:33, p, :], start=False, stop=True,
                            tile_position=(32, 0))
                        pt = pp.tile([128, 2, QW], BF16, tag="pt",
                                     name=f"pt{qb}_{p}_{j}")
                        nc.scalar.activation(
                            pt[:], ss[:],
                            mybir.ActivationFunctionType.Exp, scale=inv_sqrt)
                        if (j % 8) == 5:
                            # occasional gpsimd round to offload the DVE
                            nc.gpsimd.tensor_mul(pt[:, 0, :], pt[:, 0, :],
                                                 mt[:, j, :])
                            nc.gpsimd.tensor_mul(pt[:, 1, :], pt[:, 1, :],
                                                 mt[:, j, :])
                        else:
                            pt_b, mk_b = bass.broadcast_tensor_aps(
                                pt[:], mt[:, j:j + 1, :])
                            nc.vector.tensor_mul(pt[:], pt_b, mk_b)
                        pending.append(mk_av(qb, p, j, pt, ctx))
                        if j % 2 == 1:
                            pending.append(
                                mk_avs(qb, p, j - 1, prev_pt, pt, ctx))
                        prev_pt = pt
                        drain(nbg=2)
                    pending.extend(mk_evac(qb, p, ctx))
            while pending:
                pending.popleft()()
            bgq.extend(mk_outproj(QB - 1))
            while bgq:
                bgq.popleft()()
            _at.close()

    return nc


def _prep_inputs(q, k, v, mask, Wq, bqv, Wk, bkv, Wv, bvv, Wo):
    """Per-core input maps (numpy, host-side shard + cast)."""
    in_maps = []
    mask_h = {}
    for b in range(B):
        mt = (mask[b, 0] != 0).astype(np.float32).T  # [k, q]
        m4 = mt.reshape(KT, 128, QB, QW).transpose(2, 1, 0, 3)
        mask_h[b] = np.ascontiguousarray(
            m4.reshape(QB, 128, KT * QW)).astype(BF)

    sel2v = np.zeros((2, 128), np.float32)
    sel2v[0, 0:64] = 1.0
    sel2v[1, 64:128] = 1.0
    sel2v = sel2v.astype(BF)

    def pack_x8(x):
        # [S, D] -> fp8 [4, 128, 2, S] d-pair interleave
        xt = np.ascontiguousarray(x.T)  # [D, S]
        x8 = xt.reshape(4, 2, 128, S).transpose(0, 2, 1, 3)
        return np.ascontiguousarray(x8).astype(F8)

    def pack_w8(Wt):
        # Wt [D, PC] -> pair-packed fp8 x64: [4, 128, 2, 4, 128] + [4,128,2,8]
        r = Wt.reshape(D, HPC, DH)
        wP = np.stack([np.concatenate([r[:, 2 * p, :64],
                                       r[:, 2 * p + 1, :64]], axis=1)
                       for p in range(4)], axis=1)  # [D, 4, 128]
        w64 = r[:, :, 64]  # [D, 8]
        wP8 = (wP * W8SCALE).reshape(4, 2, 128, 4, 128).transpose(
            0, 2, 1, 3, 4)
        w648p = np.zeros((D, 16), np.float32)
        w648p[:, :HPC] = w64 * W8SCALE
        w648 = w648p.reshape(4, 2, 128, 16).transpose(0, 2, 1, 3)
        return (np.ascontiguousarray(wP8).astype(F8),
                np.ascontiguousarray(w648).astype(F8))

    def pack_bias(bv_):
        br = bv_.reshape(HPC, DH)
        bP = np.stack([np.concatenate([br[2 * p, :64], br[2 * p + 1, :64]])
                       for p in range(4)], axis=1)  # [128, 4]
        b64 = br[:, 64:65]
        return (np.ascontiguousarray(bP).astype(np.float32),
                np.ascontiguousarray(b64).astype(np.float32))

    for c in range(N_CORES):
        b, hh = c // 2, c % 2
        sl = slice(hh * PC, (hh + 1) * PC)
        WqT = np.ascontiguousarray(Wq[sl, :].T)
        WkT = np.ascontiguousarray(Wk[sl, :].T)
        wq8, wq648 = pack_w8(WqT)
        wk8, wk648 = pack_w8(WkT)
        bqP, bq64 = pack_bias(bqv[sl])
        bkP, bk64 = pack_bias(bkv[sl])
        rv = np.ascontiguousarray(Wv[sl, :].T).reshape(D, HPC, DH)
        bvr = bvv[sl].reshape(HPC, DH)
        # woT pair-major: rows 128p+64a+d = Wo[:, (2p+a)*65+d] for d<64;
        # straggler rows: head h = Wo[:, h*65+64]
        WoT = np.ascontiguousarray(Wo[:, sl].T)  # [PC, D]
        rw = WoT.reshape(HPC, DH, D)
        woP = np.concatenate(
            [np.concatenate([rw[2 * p, :64], rw[2 * p + 1, :64]], axis=0)
             for p in range(4)], axis=0)  # [512, D]
        wo4 = rw[:, 64, :]  # [8, D]
        in_maps.append({
            "xq8": pack_x8(q[b]),
            "xk8": pack_x8(k[b]),
            "xv": np.ascontiguousarray(v[b].T).astype(BF),
            "maskH": mask_h[b],
            "wq8": wq8, "wq648": wq648, "bqP": bqP, "bq64": bq64,
            "wk8": wk8, "wk648": wk648, "bkP": bkP, "bk64": bk64,
            "wvP": np.ascontiguousarray(
                rv[:, :, :64].reshape(D, 512)).astype(BF),
            "wv64": np.ascontiguousarray(rv[:, :, 64]).astype(BF),
            "bvP": bvr[:, :64].reshape(1, 512).astype(BF),
            "bv64": bvr[:, 64].reshape(1, HPC).astype(BF),
            "woP": np.ascontiguousarray(woP).astype(BF),
            "wo4": np.ascontiguousarray(wo4).astype(BF),
            "sel2": sel2v,
        })
    return in_maps


def run_sharded(in_maps, **kwargs):
    if "nc" not in _BUILT:
        _BUILT["nc"] = _build_nc()
    return run_bass_kernel_spmd(_BUILT["nc"], in_maps,
                                core_ids=list(range(N_CORES)), **kwargs)


def kernel(q, k, v, mask, Wq, bq, Wk, bk, Wv, bv, Wo, bo):
    q = np.asarray(q, np.float32)
    k = np.asarray(k, np.float32)
    v = np.asarray(v, np.float32)
    mask = np.asarray(mask)
    in_maps = _prep_inputs(q, k, v, mask,
                           np.asarray(Wq, np.float32), np.asarray(bq, np.float32),
                           np.asarray(Wk, np.float32), np.asarray(bk, np.float32),
                           np.asarray(Wv, np.float32), np.asarray(bv, np.float32),
                           np.asarray(Wo, np.float32))
    res = run_sharded(in_maps)
    bo32 = np.asarray(bo, np.float32)
    out = np.empty((B, S, D), np.float32)
    for b in range(B):
        out[b] = (res.results[2 * b]["out"].astype(np.float32)
                  + res.results[2 * b + 1]["out"].astype(np.float32) + bo32)
    return out


# revision 20
# speedup vs baseline: 1.1369x; 1.1369x over previous
"""Multi-head attention (B=4,S=2048,D=1024,H=16,dh=65) on 8 TRN2 NeuronCores.

Sharding: batch x head-half. Core c handles batch c//2 and heads
(c%2)*8..(c%2)*8+8 (P-slice of 520). Each core computes its QKV projections,
attention, and a partial out-projection; the host sums the two partials per
batch and adds bo.

v5: keep the PE at full clock. On this silicon, matmuls with a partial
contraction dim (K<128) or an explicit tile_position do not register as PE
activity for the clock governor, so sustained streams of them run at half
clock (~460ns per N=512 matmul vs ~230ns). Layout therefore:
 - K/Q projections in fp8 DoubleRow (weights x64 pre-scaled; un-scale rides
   the bias tensor_scalar), emitted per-head as zero-padded [128, S] tiles:
   rows 0-64 = the head's 65 dims (dh64 straggler row included), rows 65-127
   = zeros. Scores are then plain full-K=128 matmuls.
 - AV: plain [128, 66] stationary per head (dh64 + trailing ones column for
   the softmax row-sum), K=128 - counts as busy, runs warm.
 - Softmax: one [128, 2(heads), 512] exp per (pair, ktile) round on ScalarE
   (the ~293us floor the schedule targets), double-buffered score PSUM so PE
   never waits on the exp.
 - Mask multiply broadcast over the head dim on DVE (2x bf16), every 4th
   round on GpSimd.
 - Output written bf16, upcast + partial-sum on host.
"""

import math
import sys
from collections import deque
from contextlib import ExitStack

import numpy as np
import ml_dtypes

sys.path.insert(0, "/opt/trn_rl_repo")

import concourse.bass as bass
import concourse.mybir as mybir
import concourse.tile as tile_mod
from concourse.bass_utils import run_bass_kernel_spmd
from concourse.vector_clock import ScopedClock

# ---------------------------------------------------------------------------
# Patch for this container's walrus build: it rejects instructions carrying
# more than one semaphore wait ("Too many sync wait commands"), but Tile's
# wait assigner freely attaches several. Split excess waits onto bass_nofuse
# InstNoOp carriers on the same engine, committed immediately before the
# instruction (same-engine program order => over-synchronization only).
# ---------------------------------------------------------------------------
_MAX_WAITS = 1

_orig_commit = tile_mod.TileContext._commit_instruction


def _split_waits(self, inst, commit):
    si = inst.sync_info
    if si is None or len(si.on_wait) <= _MAX_WAITS:
        return
    waits = list(si.on_wait)
    sem_w = [w for w in waits if getattr(w, "sync_type", "semaphore") == "semaphore"]
    other_w = [w for w in waits if getattr(w, "sync_type", "semaphore") != "semaphore"]
    keep_budget = _MAX_WAITS - len(other_w)
    if keep_budget < 0:
        return
    keep = other_w + (sem_w[-keep_budget:] if keep_budget > 0 else [])
    excess = sem_w[: len(sem_w) - max(keep_budget, 0)]
    if not excess:
        return
    for i, w in enumerate(excess):
        nop = mybir.InstNoOp(
            name=f"{inst.name}-sw{i}",
            sync_info=mybir.SyncInfo(on_wait=[w], on_update=[]),
            bass_nofuse=True,
            engine=inst.engine,
        )
        commit(nop)
    inst.sync_info = mybir.SyncInfo(on_wait=keep, on_update=list(si.on_update))


def _patched_commit(self, inst, lazy_reg_writes: bool = True):
    if inst.engine != mybir.EngineType.Unassigned:
        _split_waits(self, inst, lambda n: _orig_commit(self, n, False))
    return _orig_commit(self, inst, lazy_reg_writes)


def _patched_drain_and_barrier(self, tick_clock, wait_clock):
    drain_inst = self.nc.sync.drain()
    wait_clock.add_sem_waits(
        drain_inst.ins, ScopedClock({None: tick_clock.global_clock})
    )
    si = drain_inst.ins.sync_info
    if si is not None and len(si.on_wait) > _MAX_WAITS:
        waits = list(si.on_wait)
        drain_inst.ins.sync_info = mybir.SyncInfo(
            on_wait=waits[:_MAX_WAITS], on_update=list(si.on_update)
        )
        for w in waits[_MAX_WAITS:]:
            n = self.nc.sync.nop(nofuse=True)
            n.ins.sync_info = mybir.SyncInfo(on_wait=[w], on_update=[])
    self.nc.all_engine_barrier()
    popped = self.nc._tile_sem_poison_stack.pop()
    assert popped is self._sem_poison
    self.nc.clear_and_free_semaphores(list(self.sems.allocated().values()))
    self.nc.all_engine_barrier()


tile_mod.TileContext._commit_instruction = _patched_commit
tile_mod.TileContext._drain_and_barrier = _patched_drain_and_barrier

# ---------------------------------------------------------------------------

B, S, D, H = 4, 2048, 1024, 16
DH = D // H + 1          # 65
P = H * DH               # 1040
HPC = H // 2             # heads per core
PC = HPC * DH            # 520, per-core P slice
NPAIR = HPC // 2         # 4 head pairs per core
N_CORES = 8

MT = 16                  # k tiles
KT = 16
QB = 4                   # q blocks of 512
QW = 512
AV_LAG = 2

F32 = mybir.dt.float32
BF16 = mybir.dt.bfloat16
FP8 = mybir.dt.float8e4
BF = ml_dtypes.bfloat16
F8 = ml_dtypes.float8_e4m3
W8SCALE = 64.0

# packed 128-row k-tile ranges of the 520-row concatT / WoT
PKT = [(0, 128), (128, 256), (256, 384), (384, 512), (512, 520)]

_BUILT = {}


def _build_nc():
    nc = bass.Bass("TRN2", target_bir_lowering=False, debug=False,
                   num_devices=N_CORES)
    MUL = mybir.AluOpType.mult
    ADD = mybir.AluOpType.add
    DR = mybir.MatmulPerfMode.DoubleRow

    xq8_d = nc.dram_tensor("xq8", [4, 128, 2, S], FP8, kind="ExternalInput").ap()
    xk8_d = nc.dram_tensor("xk8", [4, 128, 2, S], FP8, kind="ExternalInput").ap()
    xv_d = nc.dram_tensor("xv", [D, S], BF16, kind="ExternalInput").ap()
    mh = nc.dram_tensor("maskH", [QB, 128, KT * QW], BF16,
                        kind="ExternalInput").ap()
    wq8_d = nc.dram_tensor("wq8", [4, 128, 2, 4, 128], FP8,
                           kind="ExternalInput").ap()
    wk8_d = nc.dram_tensor("wk8", [4, 128, 2, 4, 128], FP8,
                           kind="ExternalInput").ap()
    wq648_d = nc.dram_tensor("wq648", [4, 128, 2, 16], FP8,
                             kind="ExternalInput").ap()
    wk648_d = nc.dram_tensor("wk648", [4, 128, 2, 16], FP8,
                             kind="ExternalInput").ap()
    bqP_d = nc.dram_tensor("bqP", [128, 4], F32, kind="ExternalInput").ap()
    bkP_d = nc.dram_tensor("bkP", [128, 4], F32, kind="ExternalInput").ap()
    bq64_d = nc.dram_tensor("bq64", [HPC, 1], F32, kind="ExternalInput").ap()
    bk64_d = nc.dram_tensor("bk64", [HPC, 1], F32, kind="ExternalInput").ap()
    wvP_d = nc.dram_tensor("wvP", [D, 512], BF16, kind="ExternalInput").ap()
    wv64_d = nc.dram_tensor("wv64", [D, HPC], BF16, kind="ExternalInput").ap()
    bvP_d = nc.dram_tensor("bvP", [1, 512], BF16, kind="ExternalInput").ap()
    bv64_d = nc.dram_tensor("bv64", [1, HPC], BF16, kind="ExternalInput").ap()
    wo_d = nc.dram_tensor("woT", [PC, D], BF16, kind="ExternalInput").ap()
    out = nc.dram_tensor("out", [S, D], BF16, kind="ExternalOutput").ap()

    inv_sqrt = 1.0 / math.sqrt(float(DH))

    with tile_mod.TileContext(nc) as tc:
        with tc.tile_pool(name="const", bufs=1) as pconst, \
             tc.tile_pool(name="kqT", bufs=1) as pkq, \
             tc.tile_pool(name="vh", bufs=MT) as pvh, \
             tc.tile_pool(name="pm", bufs=2) as pm, \
             tc.tile_pool(name="psS", bufs=2, space="PSUM") as psS, \
             tc.tile_pool(name="psA", bufs=1, space="PSUM") as psA, \
             tc.tile_pool(name="psO", bufs=2, space="PSUM") as psO:
            _xw = ExitStack()
            px = _xw.enter_context(tc.tile_pool(name="px", bufs=8))
            pw = _xw.enter_context(tc.tile_pool(name="pw", bufs=1))

            ones_col = pconst.tile([1, 128], BF16, tag="ones")
            nc.gpsimd.memset(ones_col[:], 1.0)
            # preload the exp table set during the projection phase
            scratch_e = pconst.tile([1, 128], BF16, tag="scratch_e")
            nc.scalar.activation(scratch_e[:], ones_col[:],
                                 mybir.ActivationFunctionType.Exp)

            # per-head K/Q tiles [128, S]: rows 0-64 = head dims (row 64 =
            # dh64 straggler), rows 65-127 zeroed -> scores run full-K=128
            kT = [pkq.tile([128, S], BF16, tag=f"kT{h}", name=f"kT{h}")
                  for h in range(HPC)]
            qT = [pkq.tile([128, S], BF16, tag=f"qT{h}", name=f"qT{h}")
                  for h in range(HPC)]
            for h in range(HPC):
                nc.gpsimd.memset(kT[h][64:128, :], 0.0)
                nc.gpsimd.memset(qT[h][64:128, :], 0.0)

            # v k-tiles with trailing ones column: [128, head, 65+1]
            vh = [pvh.tile([128, HPC, DH + 1], BF16, tag="vh", name=f"vh{j}")
                  for j in range(MT)]
            for j in range(MT):
                nc.gpsimd.memset(vh[j][:, :, DH:DH + 1], 1.0)

            # ---------------- input DMAs ------------------
            xk8 = [px.tile([128, 2, S], FP8, tag="x", name=f"xk8{dp}")
                   for dp in range(4)]
            for dp in range(4):
                nc.sync.dma_start(xk8[dp][:], xk8_d[dp])
            wk8 = [pw.tile([128, 2, 4, 128], FP8, tag=f"wk8{dp}",
                           name=f"wk8{dp}") for dp in range(4)]
            wq8 = [pw.tile([128, 2, 4, 128], FP8, tag=f"wq8{dp}",
                           name=f"wq8{dp}") for dp in range(4)]
            wk648 = [pw.tile([128, 2, 16], FP8, tag=f"wk648{dp}",
                             name=f"wk648{dp}") for dp in range(4)]
            wq648 = [pw.tile([128, 2, 16], FP8, tag=f"wq648{dp}",
                             name=f"wq648{dp}") for dp in range(4)]
            for dp in range(4):
                nc.gpsimd.dma_start(wk8[dp][:], wk8_d[dp])
                nc.gpsimd.dma_start(wk648[dp][:], wk648_d[dp])
            bkP_t = pconst.tile([128, 4], F32, tag="bkP")
            bqP_t = pconst.tile([128, 4], F32, tag="bqP")
            bk64_t = pconst.tile([HPC, 1], F32, tag="bk64")
            bq64_t = pconst.tile([HPC, 1], F32, tag="bq64")
            nc.gpsimd.dma_start(bkP_t[:], bkP_d)
            nc.gpsimd.dma_start(bqP_t[:], bqP_d)
            nc.gpsimd.dma_start(bk64_t[:], bk64_d)
            nc.gpsimd.dma_start(bq64_t[:], bq64_d)

            # -------- K/Q projection (fp8 DoubleRow), per-head evac --------
            def kq_proj(w8, w648, x8, bP, b64, kqt, tag):
                for p in range(NPAIR):
                    for c in range(4):
                        ps = psO.tile([128, QW], F32, tag="po",
                                      name=f"ps{tag}{p}_{c}")
                        for dp in range(4):
                            nc.tensor.matmul(
                                ps[:], w8[dp][:, :, p, :],
                                x8[dp][:, :, c * QW:(c + 1) * QW],
                                start=(dp == 0), stop=(dp == 3),
                                perf_mode=DR)
                        ktmp = pw.tile([128, QW], BF16, tag="ktmp",
                                       bufs=3, name=f"kt{tag}{p}_{c}")
                        nc.vector.tensor_scalar(
                            ktmp[:], ps[:],
                            1.0 / W8SCALE, bP[:, p:p + 1], MUL, ADD)
                        # split pair rows into the two per-head tiles
                        nc.sync.dma_start(
                            kqt[2 * p][0:64, c * QW:(c + 1) * QW],
                            ktmp[0:64, :])
                        nc.sync.dma_start(
                            kqt[2 * p + 1][0:64, c * QW:(c + 1) * QW],
                            ktmp[64:128, :])
                # dh64 straggler rows -> row 64 of each head tile
                for c in range(4):
                    ps8 = psO.tile([128, QW], F32, tag="po",
                                   name=f"ps8{tag}{c}")
                    for dp in range(4):
                        nc.tensor.matmul(
                            ps8[0:16, :], w648[dp][:],
                            x8[dp][:, :, c * QW:(c + 1) * QW],
                            start=(dp == 0), stop=(dp == 3), perf_mode=DR)
                    s8 = pw.tile([HPC, QW], BF16, tag="s8tmp",
                                 bufs=2, name=f"s8{tag}{c}")
                    nc.vector.tensor_scalar(
                        s8[:], ps8[0:HPC, :],
                        1.0 / W8SCALE, b64[:, 0:1], MUL, ADD)
                    for h in range(HPC):
                        nc.gpsimd.dma_start(
                            kqt[h][64:65, c * QW:(c + 1) * QW],
                            s8[h:h + 1, :])

            kq_proj(wk8, wk648, xk8, bkP_t, bk64_t, kT, "k")

            xq8 = [px.tile([128, 2, S], FP8, tag="x", name=f"xq8{dp}")
                   for dp in range(4)]
            for dp in range(4):
                nc.sync.dma_start(xq8[dp][:], xq8_d[dp])
                nc.gpsimd.dma_start(wq8[dp][:], wq8_d[dp])
                nc.gpsimd.dma_start(wq648[dp][:], wq648_d[dp])
            kq_proj(wq8, wq648, xq8, bqP_t, bq64_t, qT, "q")

            # ---------------- V projection (bf16, s-major) ----------------
            wvP_t = []
            wv64_t = []
            for d in range(8):
                wt = pw.tile([128, 512], BF16, tag="wvP", bufs=8,
                             name=f"wvP{d}")
                nc.gpsimd.dma_start(wt[:], wvP_d[d * 128:(d + 1) * 128])
                wvP_t.append(wt)
                w64t = pw.tile([128, HPC], BF16, tag="wv64", bufs=8,
                               name=f"wv64{d}")
                nc.gpsimd.dma_start(w64t[:], wv64_d[d * 128:(d + 1) * 128])
                wv64_t.append(w64t)
            bvP_t = pconst.tile([1, 512], BF16, tag="bvP")
            nc.gpsimd.dma_start(bvP_t[:], bvP_d)
            bv64_t = pconst.tile([1, HPC], BF16, tag="bv64")
            nc.gpsimd.dma_start(bv64_t[:], bv64_d)
            xv_t = [px.tile([128, S], BF16, tag="x", name=f"xv{d}")
                    for d in range(8)]
            for c in range(2):
                for d in range(8):
                    nc.sync.dma_start(
                        xv_t[d][:, c * 1024:(c + 1) * 1024],
                        xv_d[d * 128:(d + 1) * 128,
                             c * 1024:(c + 1) * 1024])

            for m in range(MT):
                psa = psA.tile([128, QW], F32, tag="avA", name=f"psva{m}")
                psb = psO.tile([128, QW], F32, tag="po", name=f"psvb{m}")
                for d in range(8):
                    nc.tensor.matmul(
                        psa[:], xv_t[d][:, m * 128:(m + 1) * 128],
                        wvP_t[d][:], start=(d == 0), stop=False)
                    nc.tensor.matmul(
                        psb[:, 0:HPC], xv_t[d][:, m * 128:(m + 1) * 128],
                        wv64_t[d][:], start=(d == 0), stop=False)
                nc.tensor.matmul(psa[:], ones_col[0:1, :],
                                 bvP_t[0:1, :], start=False, stop=True)
                nc.tensor.matmul(psb[:, 0:HPC], ones_col[0:1, :],
                                 bv64_t[0:1, :], start=False, stop=True)
                nc.vector.tensor_copy(vh[m][:, :, 0:64], psa[:])
                nc.vector.tensor_copy(vh[m][:, :, 64:65], psb[:, 0:HPC])

            _xw.close()   # free x/w pools before attention phase

            # ---------------- attention + out-projection -------------------
            _at = ExitStack()
            pp = _at.enter_context(tc.tile_pool(name="pp", bufs=8))
            pc = _at.enter_context(tc.tile_pool(name="pc", bufs=1))
            pwo = _at.enter_context(tc.tile_pool(name="pwo", bufs=1))
            pe = _at.enter_context(tc.tile_pool(name="pe", bufs=1))
            psB = _at.enter_context(
                tc.tile_pool(name="psB", bufs=1, space="PSUM"))
            # packed concatT: 128-row tiles covering rows 0..520 (head-major)
            ccp = [pc.tile([b - a, S], BF16, tag=f"ccp{i}", name=f"ccp{i}")
                   for i, (a, b) in enumerate(PKT)]
            wop = []
            for i, (a, b) in enumerate(PKT):
                w = pwo.tile([b - a, D], BF16, tag=f"wop{i}", name=f"wop{i}")
                nc.gpsimd.dma_start(w[:], wo_d[a:b, :])
                wop.append(w)

            pending = deque()
            bgq = deque()

            def drain(nbg=1):
                pops = 0
                while len(pending) > AV_LAG and pops < 4:
                    pending.popleft()()
                    pops += 1
                for _ in range(nbg):
                    if bgq:
                        bgq.popleft()()

            def mk_av(qb, p, j, pt, ctx):
                def cl():
                    if "avA" not in ctx:
                        ctx["avA"] = psA.tile([128, QW], F32, tag="avA",
                                              name=f"avA{qb}_{p}")
                        ctx["avB"] = psB.tile([128, QW], F32, tag="avB",
                                              name=f"avB{qb}_{p}")
                    nc.tensor.matmul(
                        ctx["avA"][0:DH + 1, :], vh[j][:, 2 * p, :],
                        pt[:, 0, :], start=(j == 0), stop=(j == KT - 1))
                    nc.tensor.matmul(
                        ctx["avB"][0:DH + 1, :], vh[j][:, 2 * p + 1, :],
                        pt[:, 1, :], start=(j == 0), stop=(j == KT - 1))
                return cl

            def mk_evac(qb, p, ctx):
                cls = []

                def c_rs():
                    # rows 64:66 of each AV bank = (dh64 row, ones row-sum)
                    rsA = pe.tile([2, QW], F32, tag="rsA", bufs=2,
                                  name=f"rsA{qb}_{p}")
                    rsB = pe.tile([2, QW], F32, tag="rsB", bufs=2,
                                  name=f"rsB{qb}_{p}")
                    ctx["rsA"] = rsA
                    ctx["rsB"] = rsB
                    nc.vector.tensor_copy(rsA[:], ctx["avA"][64:66, :])
                    nc.vector.tensor_copy(rsB[:], ctx["avB"][64:66, :])

                def c_gather():
                    for hh in (0, 1):
                        rs = pe.tile([1, QW], F32, tag=f"rs{hh}", bufs=2,
                                     name=f"rs{hh}_{qb}_{p}")
                        ctx[f"rs{hh}"] = rs
                        src_t = ctx["rsA"] if hh == 0 else ctx["rsB"]
                        nc.sync.dma_start(rs[0:1, :], src_t[1:2, :])

                def c_recip():
                    for hh in (0, 1):
                        rc = pe.tile([1, QW], F32, tag=f"rc{hh}", bufs=2,
                                     name=f"rc{hh}_{qb}_{p}")
                        ctx[f"rc{hh}"] = rc
                        nc.vector.reciprocal(rc[:], ctx[f"rs{hh}"][:])

                def c_rcast():
                    for hh in (0, 1):
                        rcb = pe.tile([1, QW], BF16, tag=f"rcb{hh}", bufs=2,
                                      name=f"rcb{hh}_{qb}_{p}")
                        ctx[f"rcb{hh}"] = rcb
                        nc.vector.tensor_copy(rcb[:], ctx[f"rc{hh}"][:])

                def mk_head(hh):
                    def c_rbp():
                        key = f"rbp{hh}"
                        ctx[key] = psO.tile([128, QW], F32, tag="po",
                                            name=f"rbp{qb}_{p}_{hh}")
                        nc.tensor.matmul(ctx[key][:], ones_col[0:1, :],
                                         ctx[f"rcb{hh}"][0:1, :],
                                         start=True, stop=True)

                    def c_uov():
                        key = f"uov{hh}"
                        ctx[key] = pe.tile([DH, QW], BF16, tag="uov", bufs=4,
                                           name=f"uov{qb}_{p}_{hh}")
                        av = ctx["avA"] if hh == 0 else ctx["avB"]
                        nc.vector.tensor_copy(ctx[key][:], av[0:DH, :])

                    def c_cch():
                        cch = pe.tile([DH, QW], BF16, tag="cch", bufs=4,
                                      name=f"cch{qb}_{p}_{hh}")
                        nc.vector.tensor_mul(cch[:], ctx[f"uov{hh}"][:],
                                             ctx[f"rbp{hh}"][0:DH, :])
                        # pack into 128-row concatT tiles (DMA shifts rows)
                        h = 2 * p + hh
                        r0 = h * DH
                        for i, (a, b) in enumerate(PKT):
                            lo, hi = max(r0, a), min(r0 + DH, b)
                            if lo < hi:
                                nc.gpsimd.dma_start(
                                    ccp[i][lo - a:hi - a,
                                           qb * QW:(qb + 1) * QW],
                                    cch[lo - r0:hi - r0, :])
                    return [c_rbp, c_uov, c_cch]

                cls.extend([c_rs, c_gather, c_recip, c_rcast])
                cls.extend(mk_head(0))
                cls.extend(mk_head(1))
                return cls

            def mk_outproj(qb):
                cls = []

                def mk_mm(m, n, i, ctx):
                    def cl():
                        if n not in ctx:
                            ctx[n] = psO.tile([128, QW], F32, tag="po",
                                              name=f"pso{m}_{n}")
                        nc.tensor.matmul(
                            ctx[n][:], ccp[i][:, m * 128:(m + 1) * 128],
                            wop[i][:, n * QW:(n + 1) * QW],
                            start=(i == 0), stop=(i == len(PKT) - 1))
                    return cl

                def mk_copy(m, n, ctx):
                    def cl():
                        if "osb" not in ctx:
                            ctx["osb"] = pe.tile([128, D], BF16,
                                                 tag="osb", bufs=2,
                                                 name=f"osb{m}")
                        nc.vector.tensor_copy(
                            ctx["osb"][:, n * QW:(n + 1) * QW],
                            ctx[n][:])
                    return cl

                def mk_dma(m, ctx):
                    def cl():
                        nc.sync.dma_start(
                            out[m * 128:(m + 1) * 128, :], ctx["osb"][:])
                    return cl

                for m in range(qb * 4, (qb + 1) * 4):
                    ctx = {}
                    for n in (0, 1):
                        for i in range(len(PKT)):
                            cls.append(mk_mm(m, n, i, ctx))
                        cls.append(mk_copy(m, n, ctx))
                    cls.append(mk_dma(m, ctx))
                return cls

            mts = {}

            def ensure_qb(qb):
                if qb in mts or qb >= QB:
                    return
                mt = pm.tile([128, KT, QW], BF16, tag="mask",
                             name=f"mask{qb}")
                nc.sync.dma_start(mt[:], mh[qb])
                mts[qb] = mt

            for qb in range(QB):
                ensure_qb(qb)
                mt = mts[qb]

                for p in range(NPAIR):
                    if p == 1 and qb > 0:
                        bgq.extend(mk_outproj(qb - 1))
                    if p == 2:
                        ensure_qb(qb + 1)
                    ctx = {}
                    for j in range(KT):
                        ss = psS.tile([128, 2, QW], F32, tag="ss",
                                      name=f"ss{qb}_{p}_{j}")
                        nc.tensor.matmul(
                            ss[:, 0, :], kT[2 * p][:, j * 128:(j + 1) * 128],
                            qT[2 * p][:, qb * QW:(qb + 1) * QW],
                            start=True, stop=True)
                        nc.tensor.matmul(
                            ss[:, 1, :],
                            kT[2 * p + 1][:, j * 128:(j + 1) * 128],
                            qT[2 * p + 1][:, qb * QW:(qb + 1) * QW],
                            start=True, stop=True)
                        pt = pp.tile([128, 2, QW], BF16, tag="pt",
                                     name=f"pt{qb}_{p}_{j}")
                        nc.scalar.activation(
                            pt[:], ss[:],
                            mybir.ActivationFunctionType.Exp, scale=inv_sqrt)
                        if (j % 4) == 2:
                            nc.gpsimd.tensor_mul(pt[:, 0, :], pt[:, 0, :],
                                                 mt[:, j, :])
                            nc.gpsimd.tensor_mul(pt[:, 1, :], pt[:, 1, :],
                                                 mt[:, j, :])
                        else:
                            pt_b, mk_b = bass.broadcast_tensor_aps(
                                pt[:], mt[:, j:j + 1, :])
                            nc.vector.tensor_mul(pt[:], pt_b, mk_b)
                        pending.append(mk_av(qb, p, j, pt, ctx))
                        drain(nbg=2)
                    pending.extend(mk_evac(qb, p, ctx))
            while pending:
                pending.popleft()()
            bgq.extend(mk_outproj(QB - 1))
            while bgq:
                bgq.popleft()()
            _at.close()

    return nc


def _prep_inputs(q, k, v, mask, Wq, bqv, Wk, bkv, Wv, bvv, Wo):
    """Per-core input maps (numpy, host-side shard + cast)."""
    in_maps = []
    mask_h = {}
    for b in range(B):
        mt = (mask[b, 0] != 0).astype(np.float32).T  # [k, q]
        m4 = mt.reshape(KT, 128, QB, QW).transpose(2, 1, 0, 3)
        mask_h[b] = np.ascontiguousarray(
            m4.reshape(QB, 128, KT * QW)).astype(BF)

    def pack_x8(x):
        # [S, D] -> fp8 [4, 128, 2, S] d-pair interleave
        xt = np.ascontiguousarray(x.T)  # [D, S]
        x8 = xt.reshape(4, 2, 128, S).transpose(0, 2, 1, 3)
        return np.ascontiguousarray(x8).astype(F8)

    def pack_w8(Wt):
        # Wt [D, PC] -> pair-packed fp8 x64: [4, 128, 2, 4, 128] + [4,128,2,16]
        r = Wt.reshape(D, HPC, DH)
        wP = np.stack([np.concatenate([r[:, 2 * p, :64],
                                       r[:, 2 * p + 1, :64]], axis=1)
                       for p in range(4)], axis=1)  # [D, 4, 128]
        w64 = r[:, :, 64]  # [D, 8]
        wP8 = (wP * W8SCALE).reshape(4, 2, 128, 4, 128).transpose(
            0, 2, 1, 3, 4)
        w648p = np.zeros((D, 16), np.float32)
        w648p[:, :HPC] = w64 * W8SCALE
        w648 = w648p.reshape(4, 2, 128, 16).transpose(0, 2, 1, 3)
        return (np.ascontiguousarray(wP8).astype(F8),
                np.ascontiguousarray(w648).astype(F8))

    def pack_bias(bv_):
        br = bv_.reshape(HPC, DH)
        bP = np.stack([np.concatenate([br[2 * p, :64], br[2 * p + 1, :64]])
                       for p in range(4)], axis=1)  # [128, 4]
        b64 = br[:, 64:65]
        return (np.ascontiguousarray(bP).astype(np.float32),
                np.ascontiguousarray(b64).astype(np.float32))

    for c in range(N_CORES):
        b, hh = c // 2, c % 2
        sl = slice(hh * PC, (hh + 1) * PC)
        WqT = np.ascontiguousarray(Wq[sl, :].T)
        WkT = np.ascontiguousarray(Wk[sl, :].T)
        wq8, wq648 = pack_w8(WqT)
        wk8, wk648 = pack_w8(WkT)
        bqP, bq64 = pack_bias(bqv[sl])
        bkP, bk64 = pack_bias(bkv[sl])
        rv = np.ascontiguousarray(Wv[sl, :].T).reshape(D, HPC, DH)
        bvr = bvv[sl].reshape(HPC, DH)
        in_maps.append({
            "xq8": pack_x8(q[b]),
            "xk8": pack_x8(k[b]),
            "xv": np.ascontiguousarray(v[b].T).astype(BF),
            "maskH": mask_h[b],
            "wq8": wq8, "wq648": wq648, "bqP": bqP, "bq64": bq64,
            "wk8": wk8, "wk648": wk648, "bkP": bkP, "bk64": bk64,
            "wvP": np.ascontiguousarray(
                rv[:, :, :64].reshape(D, 512)).astype(BF),
            "wv64": np.ascontiguousarray(rv[:, :, 64]).astype(BF),
            "bvP": bvr[:, :64].reshape(1, 512).astype(BF),
            "bv64": bvr[:, 64].reshape(1, HPC).astype(BF),
            "woT": np.ascontiguousarray(Wo[:, sl].T).astype(BF),
        })
    return in_maps


def run_sharded(in_maps, **kwargs):
    if "nc" not in _BUILT:
        _BUILT["nc"] = _build_nc()
    return run_bass_kernel_spmd(_BUILT["nc"], in_maps,
                                core_ids=list(range(N_CORES)), **kwargs)


def kernel(q, k, v, mask, Wq, bq, Wk, bk, Wv, bv, Wo, bo):
    q = np.asarray(q, np.float32)
    k = np.asarray(k, np.float32)
    v = np.asarray(v, np.float32)
    mask = np.asarray(mask)
    in_maps = _prep_inputs(q, k, v, mask,
                           np.asarray(Wq, np.float32), np.asarray(bq, np.float32),
                           np.asarray(Wk, np.float32), np.asarray(bk, np.float32),
                           np.asarray(Wv, np.float32), np.asarray(bv, np.float32),
                           np.asarray(Wo, np.float32))
    res = run_sharded(in_maps)
    bo32 = np.asarray(bo, np.float32)
    out = np.empty((B, S, D), np.float32)
    for b in range(B):
        out[b] = (res.results[2 * b]["out"].astype(np.float32)
                  + res.results[2 * b + 1]["out"].astype(np.float32) + bo32)
    return out


# revision 24
# speedup vs baseline: 1.2590x; 1.1074x over previous
"""Multi-head attention (B=4,S=2048,D=1024,H=16,dh=65) on 8 TRN2 NeuronCores.

Sharding: batch x head-half. Core c handles batch c//2 and heads
(c%2)*8..(c%2)*8+8 (P-slice of 520). Each core computes its QKV projections,
attention, and a partial out-projection; the host sums the two partials per
batch and adds bo.

v5: keep the PE at full clock. On this silicon, matmuls with a partial
contraction dim (K<128) or an explicit tile_position do not register as PE
activity for the clock governor, so sustained streams of them run at half
clock (~460ns per N=512 matmul vs ~230ns). Layout therefore:
 - K/Q projections in fp8 DoubleRow (weights x64 pre-scaled; un-scale rides
   the bias tensor_scalar), emitted per-head as zero-padded [128, S] tiles:
   rows 0-64 = the head's 65 dims (dh64 straggler row included), rows 65-127
   = zeros. Scores are then plain full-K=128 matmuls.
 - AV: plain [128, 66] stationary per head (dh64 + trailing ones column for
   the softmax row-sum), K=128 - counts as busy, runs warm.
 - Softmax: one [128, 2(heads), 512] exp per (pair, ktile) round on ScalarE
   (the ~293us floor the schedule targets), double-buffered score PSUM so PE
   never waits on the exp.
 - Mask multiply broadcast over the head dim on DVE (2x bf16), every 4th
   round on GpSimd.
 - Output written bf16, upcast + partial-sum on host.
"""

import math
import sys
from collections import deque
from contextlib import ExitStack

import numpy as np
import ml_dtypes

sys.path.insert(0, "/opt/trn_rl_repo")

import concourse.bass as bass
import concourse.mybir as mybir
import concourse.tile as tile_mod
from concourse.bass_utils import run_bass_kernel_spmd
from concourse.vector_clock import ScopedClock

# ---------------------------------------------------------------------------
# Patch for this container's walrus build: it rejects instructions carrying
# more than one semaphore wait ("Too many sync wait commands"), but Tile's
# wait assigner freely attaches several. Split excess waits onto bass_nofuse
# InstNoOp carriers on the same engine, committed immediately before the
# instruction (same-engine program order => over-synchronization only).
# ---------------------------------------------------------------------------
_MAX_WAITS = 1

_orig_commit = tile_mod.TileContext._commit_instruction


def _split_waits(self, inst, commit):
    si = inst.sync_info
    if si is None or len(si.on_wait) <= _MAX_WAITS:
        return
    waits = list(si.on_wait)
    sem_w = [w for w in waits if getattr(w, "sync_type", "semaphore") == "semaphore"]
    other_w = [w for w in waits if getattr(w, "sync_type", "semaphore") != "semaphore"]
    keep_budget = _MAX_WAITS - len(other_w)
    if keep_budget < 0:
        return
    keep = other_w + (sem_w[-keep_budget:] if keep_budget > 0 else [])
    excess = sem_w[: len(sem_w) - max(keep_budget, 0)]
    if not excess:
        return
    for i, w in enumerate(excess):
        nop = mybir.InstNoOp(
            name=f"{inst.name}-sw{i}",
            sync_info=mybir.SyncInfo(on_wait=[w], on_update=[]),
            bass_nofuse=True,
            engine=inst.engine,
        )
        commit(nop)
    inst.sync_info = mybir.SyncInfo(on_wait=keep, on_update=list(si.on_update))


def _patched_commit(self, inst, lazy_reg_writes: bool = True):
    if inst.engine != mybir.EngineType.Unassigned:
        _split_waits(self, inst, lambda n: _orig_commit(self, n, False))
    return _orig_commit(self, inst, lazy_reg_writes)


def _patched_drain_and_barrier(self, tick_clock, wait_clock):
    drain_inst = self.nc.sync.drain()
    wait_clock.add_sem_waits(
        drain_inst.ins, ScopedClock({None: tick_clock.global_clock})
    )
    si = drain_inst.ins.sync_info
    if si is not None and len(si.on_wait) > _MAX_WAITS:
        waits = list(si.on_wait)
        drain_inst.ins.sync_info = mybir.SyncInfo(
            on_wait=waits[:_MAX_WAITS], on_update=list(si.on_update)
        )
        for w in waits[_MAX_WAITS:]:
            n = self.nc.sync.nop(nofuse=True)
            n.ins.sync_info = mybir.SyncInfo(on_wait=[w], on_update=[])
    self.nc.all_engine_barrier()
    popped = self.nc._tile_sem_poison_stack.pop()
    assert popped is self._sem_poison
    self.nc.clear_and_free_semaphores(list(self.sems.allocated().values()))
    self.nc.all_engine_barrier()


tile_mod.TileContext._commit_instruction = _patched_commit
tile_mod.TileContext._drain_and_barrier = _patched_drain_and_barrier

# ---------------------------------------------------------------------------

B, S, D, H = 4, 2048, 1024, 16
DH = D // H + 1          # 65
P = H * DH               # 1040
HPC = H // 2             # heads per core
PC = HPC * DH            # 520, per-core P slice
NPAIR = HPC // 2         # 4 head pairs per core
N_CORES = 8

MT = 16                  # k tiles
KT = 16
QB = 4                   # q blocks of 512
QW = 512
AV_LAG = 2

F32 = mybir.dt.float32
BF16 = mybir.dt.bfloat16
FP8 = mybir.dt.float8e4
BF = ml_dtypes.bfloat16
F8 = ml_dtypes.float8_e4m3
W8SCALE = 64.0

# packed 128-row k-tile ranges of the 520-row concatT / WoT
PKT = [(0, 128), (128, 256), (256, 384), (384, 512), (512, 520)]

_BUILT = {}


def _build_nc():
    nc = bass.Bass("TRN2", target_bir_lowering=False, debug=False,
                   num_devices=N_CORES)
    MUL = mybir.AluOpType.mult
    ADD = mybir.AluOpType.add
    DR = mybir.MatmulPerfMode.DoubleRow

    xq8_d = nc.dram_tensor("xq8", [4, 128, 2, S], FP8, kind="ExternalInput").ap()
    xk8_d = nc.dram_tensor("xk8", [4, 128, 2, S], FP8, kind="ExternalInput").ap()
    xv_d = nc.dram_tensor("xv", [D, S], BF16, kind="ExternalInput").ap()
    mh = nc.dram_tensor("maskH", [QB, 128, KT * QW], BF16,
                        kind="ExternalInput").ap()
    wq8_d = nc.dram_tensor("wq8", [4, 128, 2, 4, 128], FP8,
                           kind="ExternalInput").ap()
    wk8_d = nc.dram_tensor("wk8", [4, 128, 2, 4, 128], FP8,
                           kind="ExternalInput").ap()
    wq648_d = nc.dram_tensor("wq648", [4, 128, 2, 16], FP8,
                             kind="ExternalInput").ap()
    wk648_d = nc.dram_tensor("wk648", [4, 128, 2, 16], FP8,
                             kind="ExternalInput").ap()
    bqP_d = nc.dram_tensor("bqP", [128, 4], F32, kind="ExternalInput").ap()
    bkP_d = nc.dram_tensor("bkP", [128, 4], F32, kind="ExternalInput").ap()
    bq64_d = nc.dram_tensor("bq64", [HPC, 1], F32, kind="ExternalInput").ap()
    bk64_d = nc.dram_tensor("bk64", [HPC, 1], F32, kind="ExternalInput").ap()
    wvP_d = nc.dram_tensor("wvP", [D, 512], BF16, kind="ExternalInput").ap()
    wv64_d = nc.dram_tensor("wv64", [D, HPC], BF16, kind="ExternalInput").ap()
    bvP_d = nc.dram_tensor("bvP", [1, 512], BF16, kind="ExternalInput").ap()
    bv64_d = nc.dram_tensor("bv64", [1, HPC], BF16, kind="ExternalInput").ap()
    wo_d = nc.dram_tensor("woT", [PC, D], BF16, kind="ExternalInput").ap()
    out = nc.dram_tensor("out", [S, D], BF16, kind="ExternalOutput").ap()

    inv_sqrt = 1.0 / math.sqrt(float(DH))

    with tile_mod.TileContext(nc) as tc:
        with tc.tile_pool(name="const", bufs=1) as pconst, \
             tc.tile_pool(name="kqT", bufs=1) as pkq, \
             tc.tile_pool(name="vh", bufs=MT) as pvh, \
             tc.tile_pool(name="pm", bufs=2) as pm, \
             tc.tile_pool(name="psS", bufs=2, space="PSUM") as psS, \
             tc.tile_pool(name="psA", bufs=1, space="PSUM") as psA, \
             tc.tile_pool(name="psB", bufs=1, space="PSUM") as psB, \
             tc.tile_pool(name="psO", bufs=2, space="PSUM") as psO:
            _xw = ExitStack()
            px = _xw.enter_context(tc.tile_pool(name="px", bufs=8))
            pw = _xw.enter_context(tc.tile_pool(name="pw", bufs=1))

            ones_col = pconst.tile([1, 128], BF16, tag="ones")
            nc.gpsimd.memset(ones_col[:], 1.0)
            # [128,128] with ones in row 0 only: full-K broadcast stationary
            ones128 = pconst.tile([128, 128], BF16, tag="ones128")
            # preload the exp table set during the projection phase
            scratch_e = pconst.tile([1, 128], BF16, tag="scratch_e")
            nc.scalar.activation(scratch_e[:], ones_col[:],
                                 mybir.ActivationFunctionType.Exp)

            # per-head K/Q tiles [128, S]: rows 0-64 = head dims (row 64 =
            # dh64 straggler), rows 65-127 zeroed -> scores run full-K=128
            kT = [pkq.tile([128, S], BF16, tag=f"kT{h}", name=f"kT{h}")
                  for h in range(HPC)]
            qT = [pkq.tile([128, S], BF16, tag=f"qT{h}", name=f"qT{h}")
                  for h in range(HPC)]


            # v k-tiles with trailing ones column: [128, head, 65+1]
            vh = [pvh.tile([128, HPC, DH + 1], BF16, tag="vh", name=f"vh{j}")
                  for j in range(MT)]

            # ---------------- input DMAs ------------------
            xk8 = [px.tile([128, 2, S], FP8, tag="x", name=f"xk8{dp}")
                   for dp in range(4)]
            for dp in range(4):
                nc.sync.dma_start(xk8[dp][:], xk8_d[dp])
            wk8 = [pw.tile([128, 2, 4, 128], FP8, tag=f"wk8{dp}",
                           name=f"wk8{dp}") for dp in range(4)]
            wq8 = [pw.tile([128, 2, 4, 128], FP8, tag=f"wq8{dp}",
                           name=f"wq8{dp}") for dp in range(4)]
            wk648 = [pw.tile([128, 2, 16], FP8, tag=f"wk648{dp}",
                             name=f"wk648{dp}") for dp in range(4)]
            wq648 = [pw.tile([128, 2, 16], FP8, tag=f"wq648{dp}",
                             name=f"wq648{dp}") for dp in range(4)]
            for dp in range(4):
                nc.gpsimd.dma_start(wk8[dp][:], wk8_d[dp])
                nc.gpsimd.dma_start(wk648[dp][:], wk648_d[dp])
            bkP_t = pconst.tile([128, 4], F32, tag="bkP")
            bqP_t = pconst.tile([128, 4], F32, tag="bqP")
            bk64_t = pconst.tile([HPC, 1], F32, tag="bk64")
            bq64_t = pconst.tile([HPC, 1], F32, tag="bq64")
            nc.gpsimd.dma_start(bkP_t[:], bkP_d)
            nc.gpsimd.dma_start(bqP_t[:], bqP_d)
            nc.gpsimd.dma_start(bk64_t[:], bk64_d)
            nc.gpsimd.dma_start(bq64_t[:], bq64_d)

            # -------- K/Q projection (fp8 DoubleRow), per-head evac --------
            def kq_proj(w8, w648, x8, bP, b64, kqt, tag):
                for p in range(NPAIR):
                    for c in range(4):
                        ps = psO.tile([128, QW], F32, tag="po",
                                      name=f"ps{tag}{p}_{c}")
                        for dp in range(4):
                            nc.tensor.matmul(
                                ps[:], w8[dp][:, :, p, :],
                                x8[dp][:, :, c * QW:(c + 1) * QW],
                                start=(dp == 0), stop=(dp == 3),
                                perf_mode=DR)
                        ktmp = pw.tile([128, QW], BF16, tag="ktmp",
                                       bufs=3, name=f"kt{tag}{p}_{c}")
                        nc.vector.tensor_scalar(
                            ktmp[:], ps[:],
                            1.0 / W8SCALE, bP[:, p:p + 1], MUL, ADD)
                        # split pair rows into the two per-head tiles
                        nc.sync.dma_start(
                            kqt[2 * p][0:64, c * QW:(c + 1) * QW],
                            ktmp[0:64, :])
                        nc.sync.dma_start(
                            kqt[2 * p + 1][0:64, c * QW:(c + 1) * QW],
                            ktmp[64:128, :])
                # zero pad rows 65-127 (and 64, overwritten below) so the
                # score matmuls run with a full K=128 contraction
                for h in range(HPC):
                    nc.vector.memset(kqt[h][64:128, :], 0.0)
                # dh64 straggler rows -> row 64 of each head tile
                for c in range(4):
                    ps8 = psO.tile([128, QW], F32, tag="po",
                                   name=f"ps8{tag}{c}")
                    for dp in range(4):
                        nc.tensor.matmul(
                            ps8[0:16, :], w648[dp][:],
                            x8[dp][:, :, c * QW:(c + 1) * QW],
                            start=(dp == 0), stop=(dp == 3), perf_mode=DR)
                    s8 = pw.tile([HPC, QW], BF16, tag="s8tmp",
                                 bufs=2, name=f"s8{tag}{c}")
                    nc.vector.tensor_scalar(
                        s8[:], ps8[0:HPC, :],
                        1.0 / W8SCALE, b64[:, 0:1], MUL, ADD)
                    for h in range(HPC):
                        nc.gpsimd.dma_start(
                            kqt[h][64:65, c * QW:(c + 1) * QW],
                            s8[h:h + 1, :])

            xq8 = [px.tile([128, 2, S], FP8, tag="x", name=f"xq8{dp}")
                   for dp in range(4)]
            for dp in range(4):
                nc.sync.dma_start(xq8[dp][:], xq8_d[dp])
                nc.gpsimd.dma_start(wq8[dp][:], wq8_d[dp])
                nc.gpsimd.dma_start(wq648[dp][:], wq648_d[dp])

            kq_proj(wk8, wk648, xk8, bkP_t, bk64_t, kT, "k")
            kq_proj(wq8, wq648, xq8, bqP_t, bq64_t, qT, "q")

            # ---------------- V projection (bf16, s-major) ----------------
            wvP_t = []
            wv64_t = []
            for d in range(8):
                wt = pw.tile([128, 512], BF16, tag="wvP", bufs=8,
                             name=f"wvP{d}")
                nc.gpsimd.dma_start(wt[:], wvP_d[d * 128:(d + 1) * 128])
                wvP_t.append(wt)
                w64t = pw.tile([128, HPC], BF16, tag="wv64", bufs=8,
                               name=f"wv64{d}")
                nc.gpsimd.dma_start(w64t[:], wv64_d[d * 128:(d + 1) * 128])
                wv64_t.append(w64t)
            bvP_t = pconst.tile([1, 512], BF16, tag="bvP")
            nc.gpsimd.dma_start(bvP_t[:], bvP_d)
            bv64_t = pconst.tile([1, HPC], BF16, tag="bv64")
            nc.gpsimd.dma_start(bv64_t[:], bv64_d)
            xv_t = [px.tile([128, S], BF16, tag="x", name=f"xv{d}")
                    for d in range(8)]
            for c in range(2):
                for d in range(8):
                    nc.gpsimd.dma_start(
                        xv_t[d][:, c * 1024:(c + 1) * 1024],
                        xv_d[d * 128:(d + 1) * 128,
                             c * 1024:(c + 1) * 1024])

            for j in range(MT):
                nc.vector.memset(vh[j][:, :, DH:DH + 1], 1.0)
            for m in range(MT):
                # alternate the two (currently free) AV rings for 2x buffering
                psa = (psA if m % 2 == 0 else psB).tile(
                    [128, QW], F32, tag="avA" if m % 2 == 0 else "avB",
                    name=f"psva{m}")
                psb = psO.tile([128, QW], F32, tag="po", name=f"psvb{m}")
                for d in range(8):
                    nc.tensor.matmul(
                        psa[:], xv_t[d][:, m * 128:(m + 1) * 128],
                        wvP_t[d][:], start=(d == 0), stop=False)
                    nc.tensor.matmul(
                        psb[:, 0:HPC], xv_t[d][:, m * 128:(m + 1) * 128],
                        wv64_t[d][:], start=(d == 0), stop=False)
                nc.tensor.matmul(psa[:], ones_col[0:1, :],
                                 bvP_t[0:1, :], start=False, stop=True)
                nc.tensor.matmul(psb[:, 0:HPC], ones_col[0:1, :],
                                 bv64_t[0:1, :], start=False, stop=True)
                nc.vector.tensor_copy(vh[m][:, :, 0:64], psa[:])
                nc.vector.tensor_copy(vh[m][:, :, 64:65], psb[:, 0:HPC])

            _xw.close()   # free x/w pools before attention phase

            # ---------------- attention + out-projection -------------------
            _at = ExitStack()
            pp = _at.enter_context(tc.tile_pool(name="pp", bufs=8))
            pc = _at.enter_context(tc.tile_pool(name="pc", bufs=1))
            pwo = _at.enter_context(tc.tile_pool(name="pwo", bufs=1))
            pe = _at.enter_context(tc.tile_pool(name="pe", bufs=1))
            # packed concatT: 128-row tiles covering rows 0..520 (head-major)
            nc.vector.memset(ones128[:], 0.0)
            nc.vector.memset(ones128[0:1, :], 1.0)
            ccp = [pc.tile([128, S], BF16, tag=f"ccp{i}", name=f"ccp{i}")
                   for i in range(len(PKT))]
            nc.vector.memset(ccp[4][:], 0.0)  # rows 8-127 stay zero
            wop = []
            for i, (a, b) in enumerate(PKT):
                w = pwo.tile([128, D], BF16, tag=f"wop{i}", name=f"wop{i}")
                if i == 4:
                    nc.vector.memset(w[:], 0.0)
                nc.gpsimd.dma_start(w[0:b - a, :], wo_d[a:b, :])
                wop.append(w)

            pending = deque()
            bgq = deque()

            def drain(nbg=1):
                pops = 0
                while len(pending) > AV_LAG and pops < 4:
                    pending.popleft()()
                    pops += 1
                for _ in range(nbg):
                    if bgq:
                        bgq.popleft()()

            def mk_av(qb, p, j, pt, ctx):
                def cl():
                    if "avA" not in ctx:
                        ctx["avA"] = psA.tile([128, QW], F32, tag="avA",
                                              name=f"avA{qb}_{p}")
                        ctx["avB"] = psB.tile([128, QW], F32, tag="avB",
                                              name=f"avB{qb}_{p}")
                    nc.tensor.matmul(
                        ctx["avA"][0:DH + 1, :], vh[j][:, 2 * p, :],
                        pt[:, 0, :], start=(j == 0), stop=(j == KT - 1))
                    nc.tensor.matmul(
                        ctx["avB"][0:DH + 1, :], vh[j][:, 2 * p + 1, :],
                        pt[:, 1, :], start=(j == 0), stop=(j == KT - 1))
                return cl

            def mk_evac(qb, p, ctx):
                cls = []

                def c_rs():
                    # rows 64:66 of each AV bank = (dh64 row, ones row-sum)
                    rsA = pe.tile([2, QW], F32, tag="rsA", bufs=2,
                                  name=f"rsA{qb}_{p}")
                    rsB = pe.tile([2, QW], F32, tag="rsB", bufs=2,
                                  name=f"rsB{qb}_{p}")
                    ctx["rsA"] = rsA
                    ctx["rsB"] = rsB
                    nc.vector.tensor_copy(rsA[:], ctx["avA"][64:66, :])
                    nc.vector.tensor_copy(rsB[:], ctx["avB"][64:66, :])

                def c_gather():
                    rsAB = pe.tile([2, QW], F32, tag="rsAB", bufs=2,
                                   name=f"rsAB{qb}_{p}")
                    ctx["rsAB"] = rsAB
                    nc.sync.dma_start(rsAB[0:1, :], ctx["rsA"][1:2, :])
                    nc.sync.dma_start(rsAB[1:2, :], ctx["rsB"][1:2, :])

                def c_recip():
                    rc = pe.tile([2, QW], F32, tag="rc", bufs=2,
                                 name=f"rc{qb}_{p}")
                    ctx["rc"] = rc
                    nc.vector.reciprocal(rc[:], ctx["rsAB"][:])

                def c_rcast():
                    first = (qb == 0 and p < 2)
                    for hh in (0, 1):
                        rcb = pe.tile([128, QW], BF16, tag=f"rcb{hh}",
                                      bufs=2, name=f"rcb{hh}_{qb}_{p}")
                        ctx[f"rcb{hh}"] = rcb
                        if first:
                            # clear potential NaN garbage in rows 1-127 once
                            # per ring buffer; later reuses hold old finite
                            # recips which the zero stationary rows ignore
                            nc.vector.memset(rcb[:], 0.0)
                    # head A: DVE cast from rc row 0 (base-0 ok)
                    nc.vector.tensor_copy(ctx["rcb0"][0:1, :],
                                          ctx["rc"][0:1, :])
                    # head B: rc row 1 is partition base 1 - move via DMA
                    # to a base-0 f32 tile, then cast
                    rc1 = pe.tile([1, QW], F32, tag="rc1", bufs=2,
                                  name=f"rc1_{qb}_{p}")
                    nc.sync.dma_start(rc1[0:1, :], ctx["rc"][1:2, :])
                    nc.vector.tensor_copy(ctx["rcb1"][0:1, :], rc1[0:1, :])

                def mk_head(hh):
                    def c_rbp():
                        key = f"rbp{hh}"
                        ctx[key] = psO.tile([128, QW], F32, tag="po",
                                            name=f"rbp{qb}_{p}_{hh}")
                        nc.tensor.matmul(ctx[key][:], ones128[:],
                                         ctx[f"rcb{hh}"][:],
                                         start=True, stop=True)

                    def c_uov():
                        key = f"uov{hh}"
                        ctx[key] = pe.tile([DH, QW], BF16, tag="uov", bufs=4,
                                           name=f"uov{qb}_{p}_{hh}")
                        av = ctx["avA"] if hh == 0 else ctx["avB"]
                        nc.vector.tensor_copy(ctx[key][:], av[0:DH, :])

                    def c_cch():
                        cch = pe.tile([DH, QW], BF16, tag="cch", bufs=4,
                                      name=f"cch{qb}_{p}_{hh}")
                        nc.vector.tensor_mul(cch[:], ctx[f"uov{hh}"][:],
                                             ctx[f"rbp{hh}"][0:DH, :])
                        # pack into 128-row concatT tiles (DMA shifts rows)
                        h = 2 * p + hh
                        r0 = h * DH
                        for i, (a, b) in enumerate(PKT):
                            lo, hi = max(r0, a), min(r0 + DH, b)
                            if lo < hi:
                                nc.gpsimd.dma_start(
                                    ccp[i][lo - a:hi - a,
                                           qb * QW:(qb + 1) * QW],
                                    cch[lo - r0:hi - r0, :])
                    return [c_rbp, c_uov, c_cch]

                cls.extend([c_rs, c_gather, c_recip, c_rcast])
                cls.extend(mk_head(0))
                cls.extend(mk_head(1))
                return cls

            def mk_outproj(qb):
                cls = []

                def mk_mm(m, n, i, ctx):
                    def cl():
                        if n not in ctx:
                            ctx[n] = psO.tile([128, QW], F32, tag="po",
                                              name=f"pso{m}_{n}")
                        nc.tensor.matmul(
                            ctx[n][:], ccp[i][:, m * 128:(m + 1) * 128],
                            wop[i][:, n * QW:(n + 1) * QW],
                            start=(i == 0), stop=(i == len(PKT) - 1))
                    return cl

                def mk_copy(m, n, ctx):
                    def cl():
                        if "osb" not in ctx:
                            ctx["osb"] = pe.tile([128, D], BF16,
                                                 tag="osb", bufs=2,
                                                 name=f"osb{m}")
                        nc.vector.tensor_copy(
                            ctx["osb"][:, n * QW:(n + 1) * QW],
                            ctx[n][:])
                    return cl

                def mk_dma(m, ctx):
                    def cl():
                        nc.sync.dma_start(
                            out[m * 128:(m + 1) * 128, :], ctx["osb"][:])
                    return cl

                for m in range(qb * 4, (qb + 1) * 4):
                    ctx = {}
                    for n in (0, 1):
                        for i in range(len(PKT)):
                            cls.append(mk_mm(m, n, i, ctx))
                        cls.append(mk_copy(m, n, ctx))
                    cls.append(mk_dma(m, ctx))
                return cls

            mts = {}

            def ensure_qb(qb):
                if qb in mts or qb >= QB:
                    return
                mt = pm.tile([128, KT, QW], BF16, tag="mask",
                             name=f"mask{qb}")
                nc.sync.dma_start(mt[:], mh[qb])
                mts[qb] = mt

            for qb in range(QB):
                ensure_qb(qb)
                mt = mts[qb]

                for p in range(NPAIR):
                    if p == 1 and qb > 0:
                        bgq.extend(mk_outproj(qb - 1))
                    if p == 2:
                        ensure_qb(qb + 1)
                    ctx = {}
                    for j in range(KT):
                        ss = psS.tile([128, 2, QW], F32, tag="ss",
                                      name=f"ss{qb}_{p}_{j}")
                        nc.tensor.matmul(
                            ss[:, 0, :], kT[2 * p][:, j * 128:(j + 1) * 128],
                            qT[2 * p][:, qb * QW:(qb + 1) * QW],
                            start=True, stop=True)
                        nc.tensor.matmul(
                            ss[:, 1, :],
                            kT[2 * p + 1][:, j * 128:(j + 1) * 128],
                            qT[2 * p + 1][:, qb * QW:(qb + 1) * QW],
                            start=True, stop=True)
                        pt = pp.tile([128, 2, QW], BF16, tag="pt",
                                     name=f"pt{qb}_{p}_{j}")
                        nc.scalar.activation(
                            pt[:], ss[:],
                            mybir.ActivationFunctionType.Exp, scale=inv_sqrt)
                        if (j % 8) == 2:
                            nc.gpsimd.tensor_mul(pt[:, 0, :], pt[:, 0, :],
                                                 mt[:, j, :])
                            nc.gpsimd.tensor_mul(pt[:, 1, :], pt[:, 1, :],
                                                 mt[:, j, :])
                        else:
                            pt_b, mk_b = bass.broadcast_tensor_aps(
                                pt[:], mt[:, j:j + 1, :])
                            nc.vector.tensor_mul(pt[:], pt_b, mk_b)
                        pending.append(mk_av(qb, p, j, pt, ctx))
                        drain(nbg=2)
                    pending.extend(mk_evac(qb, p, ctx))
            while pending:
                pending.popleft()()
            bgq.extend(mk_outproj(QB - 1))
            while bgq:
                bgq.popleft()()
            _at.close()

    return nc


def _prep_inputs(q, k, v, mask, Wq, bqv, Wk, bkv, Wv, bvv, Wo):
    """Per-core input maps (numpy, host-side shard + cast)."""
    in_maps = []
    mask_h = {}
    for b in range(B):
        mt = (mask[b, 0] != 0).astype(np.float32).T  # [k, q]
        m4 = mt.reshape(KT, 128, QB, QW).transpose(2, 1, 0, 3)
        mask_h[b] = np.ascontiguousarray(
            m4.reshape(QB, 128, KT * QW)).astype(BF)

    def pack_x8(x):
        # [S, D] -> fp8 [4, 128, 2, S] d-pair interleave
        xt = np.ascontiguousarray(x.T)  # [D, S]
        x8 = xt.reshape(4, 2, 128, S).transpose(0, 2, 1, 3)
        return np.ascontiguousarray(x8).astype(F8)

    def pack_w8(Wt):
        # Wt [D, PC] -> pair-packed fp8 x64: [4, 128, 2, 4, 128] + [4,128,2,16]
        r = Wt.reshape(D, HPC, DH)
        wP = np.stack([np.concatenate([r[:, 2 * p, :64],
                                       r[:, 2 * p + 1, :64]], axis=1)
                       for p in range(4)], axis=1)  # [D, 4, 128]
        w64 = r[:, :, 64]  # [D, 8]
        wP8 = (wP * W8SCALE).reshape(4, 2, 128, 4, 128).transpose(
            0, 2, 1, 3, 4)
        w648p = np.zeros((D, 16), np.float32)
        w648p[:, :HPC] = w64 * W8SCALE
        w648 = w648p.reshape(4, 2, 128, 16).transpose(0, 2, 1, 3)
        return (np.ascontiguousarray(wP8).astype(F8),
                np.ascontiguousarray(w648).astype(F8))

    def pack_bias(bv_):
        br = bv_.reshape(HPC, DH)
        bP = np.stack([np.concatenate([br[2 * p, :64], br[2 * p + 1, :64]])
                       for p in range(4)], axis=1)  # [128, 4]
        b64 = br[:, 64:65]
        return (np.ascontiguousarray(bP).astype(np.float32),
                np.ascontiguousarray(b64).astype(np.float32))

    for c in range(N_CORES):
        b, hh = c // 2, c % 2
        sl = slice(hh * PC, (hh + 1) * PC)
        WqT = np.ascontiguousarray(Wq[sl, :].T)
        WkT = np.ascontiguousarray(Wk[sl, :].T)
        wq8, wq648 = pack_w8(WqT)
        wk8, wk648 = pack_w8(WkT)
        bqP, bq64 = pack_bias(bqv[sl])
        bkP, bk64 = pack_bias(bkv[sl])
        rv = np.ascontiguousarray(Wv[sl, :].T).reshape(D, HPC, DH)
        bvr = bvv[sl].reshape(HPC, DH)
        in_maps.append({
            "xq8": pack_x8(q[b]),
            "xk8": pack_x8(k[b]),
            "xv": np.ascontiguousarray(v[b].T).astype(BF),
            "maskH": mask_h[b],
            "wq8": wq8, "wq648": wq648, "bqP": bqP, "bq64": bq64,
            "wk8": wk8, "wk648": wk648, "bkP": bkP, "bk64": bk64,
            "wvP": np.ascontiguousarray(
                rv[:, :, :64].reshape(D, 512)).astype(BF),
            "wv64": np.ascontiguousarray(rv[:, :, 64]).astype(BF),
            "bvP": bvr[:, :64].reshape(1, 512).astype(BF),
            "bv64": bvr[:, 64].reshape(1, HPC).astype(BF),
            "woT": np.ascontiguousarray(Wo[:, sl].T).astype(BF),
        })
    return in_maps


def run_sharded(in_maps, **kwargs):
    if "nc" not in _BUILT:
        _BUILT["nc"] = _build_nc()
    return run_bass_kernel_spmd(_BUILT["nc"], in_maps,
                                core_ids=list(range(N_CORES)), **kwargs)


def kernel(q, k, v, mask, Wq, bq, Wk, bk, Wv, bv, Wo, bo):
    q = np.asarray(q, np.float32)
    k = np.asarray(k, np.float32)
    v = np.asarray(v, np.float32)
    mask = np.asarray(mask)
    in_maps = _prep_inputs(q, k, v, mask,
                           np.asarray(Wq, np.float32), np.asarray(bq, np.float32),
                           np.asarray(Wk, np.float32), np.asarray(bk, np.float32),
                           np.asarray(Wv, np.float32), np.asarray(bv, np.float32),
                           np.asarray(Wo, np.float32))
    res = run_sharded(in_maps)
    bo32 = np.asarray(bo, np.float32)
    out = np.empty((B, S, D), np.float32)
    for b in range(B):
        out[b] = (res.results[2 * b]["out"].astype(np.float32)
                  + res.results[2 * b + 1]["out"].astype(np.float32) + bo32)
    return out
